# revision 14
# baseline (speedup 1.0000x reference)
"""Trainium2 Bass kernel for nn_CrossAttention (single-head NxN attention + proj + InstanceNorm + residual).

Sharding: 8 cores = (batch b in 0..3) x (query-half h in 0..1).
Each core computes its half of the query tokens for one batch; the
InstanceNorm statistics (over the full 4096 tokens) are combined across
the core pair with a tiny AllGather.

Precision: matmul operands in fp16 (safe: every tensor here has tiny
dynamic range - |scores|<~6, p=exp(s)<~250, |q|,|k|,|v|<~6), all
accumulation (PSUM), softmax denominators, InstanceNorm statistics and
the residual path in fp32.  End-to-end relative RMS error ~4e-4.

Self-contained: hardcodes shapes B=4, C=256, D=H=W=16 (N=4096), Cr=32.
"""

import numpy as np

import concourse.bass as bass
import concourse.mybir as mybir
import concourse.tile as tile
from concourse import bacc
from concourse.bass_utils import run_bass_kernel_spmd
from concourse.masks import make_identity

B, C, N, Cr = 4, 256, 4096, 32
NH = N // 2  # query tokens per core
EPS = 1e-5
SCALE = float(Cr) ** -0.5
FP32 = mybir.dt.float32
FP16 = mybir.dt.float16

N_CORES = 8
REPLICA_GROUPS = [[0, 1], [2, 3], [4, 5], [6, 7]]

IT = 512                   # i-tile width (query columns processed together)
N_ITILES = NH // IT        # 4
JBLK = 128                 # j-block (rows per QK matmul output)
N_JBLK = N // JBLK         # 32
JB_PER_BURST = 2           # j-blocks per burst; each row-tiled QK matmul owns a full PSUM bank
N_JBURSTS = N_JBLK // JB_PER_BURST  # 16

AF = mybir.ActivationFunctionType
ALU = mybir.AluOpType

LAST_RESULTS = None  # BassKernelResults of the most recent run (for test harness)


def build_nc(use_collective=True):
    nc = bacc.Bacc("TRN2", num_devices=N_CORES, name="xattn",
                   target_bir_lowering=False)

    x1h_d = nc.dram_tensor("x1h", [C, NH], FP32, kind="ExternalInput").ap()
    x2b_d = nc.dram_tensor("x2b", [C, N], FP16, kind="ExternalInput").ap()
    wqT_d = nc.dram_tensor("wqT", [C, Cr], FP16, kind="ExternalInput").ap()
    wkT_d = nc.dram_tensor("wkT", [C, Cr], FP16, kind="ExternalInput").ap()
    wvT_d = nc.dram_tensor("wvT", [C, C], FP16, kind="ExternalInput").ap()
    wpT_d = nc.dram_tensor("wpT", [C, C], FP16, kind="ExternalInput").ap()
    out_d = nc.dram_tensor("out", [C, NH], FP32, kind="ExternalOutput").ap()

    with tile.TileContext(nc) as tc:
        build_body(tc, x1h_d, x2b_d, wqT_d, wkT_d, wvT_d, wpT_d, out_d,
                   use_collective)
    nc.compile()
    return nc


def build_body(tc, x1h_d, x2b_d, wqT_d, wkT_d, wvT_d, wpT_d, out_d,
               use_collective=True):
    nc = tc.nc
    from contextlib import ExitStack

    with ExitStack() as ctx:
        persist = ctx.enter_context(tc.tile_pool(name="persist", bufs=1))
        ptp = ctx.enter_context(tc.tile_pool(name="ptp", bufs=3))
        sm = ctx.enter_context(tc.tile_pool(name="sm", bufs=4))
        sm2 = ctx.enter_context(tc.tile_pool(name="sm2", bufs=2))
        qkp = ctx.enter_context(tc.tile_pool(name="qkp", bufs=2, space="PSUM"))
        avp = ctx.enter_context(tc.tile_pool(name="avp", bufs=4, space="PSUM"))
        dramp = ctx.enter_context(tc.tile_pool(name="dramp", bufs=1, space="DRAM"))

        # ---- constants -------------------------------------------------
        eps_sb = persist.tile([128, 1], FP32, tag="eps", name="eps_sb")
        nc.vector.memset(eps_sb, EPS)
        ident = persist.tile([128, 128], FP32, tag="ident", name="ident")
        make_identity(nc, ident)

        # ---- loads: x2 + wv first (vT matmuls start earliest) ----------
        x2_sb = [persist.tile([128, N], FP16, tag=f"x2_{cc}", name=f"x2_sb{cc}")
                 for cc in range(2)]
        wv_sb = [persist.tile([128, C], FP16, tag=f"wv{cc}", name=f"wv_sb{cc}")
                 for cc in range(2)]
        for ch in range(4):
            sl = slice(1024 * ch, 1024 * (ch + 1))
            for cc in range(2):
                nc.sync.dma_start(x2_sb[cc][:, sl],
                                  x2b_d[128 * cc:128 * (cc + 1), sl])
            if ch == 0:
                for cc in range(2):
                    nc.sync.dma_start(wv_sb[cc], wvT_d[128 * cc:128 * (cc + 1), :])
        wq_sb, wk_sb, wp_sb = [], [], []
        for cc in range(2):
            w2 = persist.tile([128, Cr], FP16, tag=f"wk{cc}", name=f"wk_sb{cc}")
            nc.sync.dma_start(w2, wkT_d[128 * cc:128 * (cc + 1), :])
            wk_sb.append(w2)
            w1 = persist.tile([128, Cr], FP16, tag=f"wq{cc}", name=f"wq_sb{cc}")
            nc.sync.dma_start(w1, wqT_d[128 * cc:128 * (cc + 1), :])
            wq_sb.append(w1)
            w4 = persist.tile([128, C], FP16, tag=f"wp{cc}", name=f"wp_sb{cc}")
            nc.sync.dma_start(w4, wpT_d[128 * cc:128 * (cc + 1), :])
            wp_sb.append(w4)
        x1_sb, x1_hf = [], []
        for cc in range(2):
            t = persist.tile([128, NH], FP32, tag=f"x1_{cc}", name=f"x1_sb{cc}")
            for ch in range(2):
                sl = slice(1024 * ch, 1024 * (ch + 1))
                nc.sync.dma_start(t[:, sl], x1h_d[128 * cc:128 * (cc + 1), sl])
            x1_sb.append(t)
            tb = persist.tile([128, NH], FP16, tag=f"x1f_{cc}", name=f"x1_hf{cc}")
            nc.vector.tensor_copy(tb, t)
            x1_hf.append(tb)

        # ---- vT = x2^T @ wv^T -> [j, c] fp16 with ones column ----------
        vt = persist.tile([128, N_JBLK, C + 1], FP16, tag="vt", name="vt")
        nc.vector.memset(vt[:, :, C:C + 1], 1.0)
        for jblk in range(N_JBLK):
            vp = avp.tile([128, C], FP32, tag="av", name=f"vp{jblk}")
            for cc in range(2):
                nc.tensor.matmul(
                    vp, lhsT=x2_sb[cc][:, 128 * jblk:128 * (jblk + 1)],
                    rhs=wv_sb[cc], start=(cc == 0), stop=(cc == 1))
            nc.scalar.copy(out=vt[:, jblk, 0:C], in_=vp)

        # ---- k, q replicated across the four 32-partition groups ------
        k_rep = persist.tile([128, N], FP16, tag="krep", name="k_rep")
        for jt in range(8):
            kp = avp.tile([128, 512], FP32, tag="av", name=f"kp{jt}")
            for ct in range(4):
                for cc in range(2):
                    nc.tensor.matmul(
                        kp[32 * ct:32 * (ct + 1), :], lhsT=wk_sb[cc],
                        rhs=x2_sb[cc][:, 512 * jt:512 * (jt + 1)],
                        start=(cc == 0), stop=(cc == 1),
                        tile_position=(0, 32 * ct))
            nc.vector.tensor_copy(k_rep[:, 512 * jt:512 * (jt + 1)], kp)

        q_rep = persist.tile([128, NH], FP16, tag="qrep", name="q_rep")
        for qt in range(4):
            qp = avp.tile([128, 512], FP32, tag="av", name=f"qp{qt}")
            for ct in range(4):
                for cc in range(2):
                    nc.tensor.matmul(
                        qp[32 * ct:32 * (ct + 1), :], lhsT=wq_sb[cc],
                        rhs=x1_hf[cc][:, 512 * qt:512 * (qt + 1)],
                        start=(cc == 0), stop=(cc == 1),
                        tile_position=(0, 32 * ct))
            nc.vector.tensor_copy(q_rep[:, 512 * qt:512 * (qt + 1)], qp)

        # ---- persistent attention outputs ------------------------------
        proj_sb = [persist.tile([128, NH], FP32, tag=f"proj{ob}", name=f"proj_sb{ob}")
                   for ob in range(2)]
        stats_sb = [persist.tile([128, N_ITILES, 6], FP32, tag=f"stats{ob}",
                                 name=f"stats_sb{ob}") for ob in range(2)]

        def emit_qk(it, jb):
            isl = slice(IT * it, IT * (it + 1))
            qk = qkp.tile([128, IT * JB_PER_BURST], FP32, tag="qk",
                          name=f"qk{it}_{jb}")
            for t in range(JB_PER_BURST):
                jblk = jb * JB_PER_BURST + t
                rt = t + 2 * (jb % 2)   # alternate row-groups between bursts
                nc.tensor.matmul(
                    qk[:, IT * t:IT * (t + 1)],
                    lhsT=k_rep[32 * rt:32 * (rt + 1),
                               JBLK * jblk:JBLK * (jblk + 1)],
                    rhs=q_rep[32 * rt:32 * (rt + 1), isl],
                    start=True, stop=True, tile_position=(32 * rt, 0))
            pt = ptp.tile([128, IT * JB_PER_BURST], FP16, tag="pt",
                          name=f"pt{it}_{jb}")
            nc.scalar.activation(out=pt, in_=qk, func=AF.Exp)
            return pt

        def emit_av(av_t, jb, pt):
            for t in range(JB_PER_BURST):
                jblk = jb * JB_PER_BURST + t
                for ib in range(4):
                    nc.tensor.matmul(
                        av_t[ib],
                        lhsT=pt[:, IT * t + 128 * ib:IT * t + 128 * (ib + 1)],
                        rhs=vt[:, jblk, :],
                        start=(jb == 0 and t == 0),
                        stop=(jb == N_JBURSTS - 1 and t == JB_PER_BURST - 1))

        def emit_epilogue(it, av_t):
            isl = slice(IT * it, IT * (it + 1))
            avc = [sm2.tile([128, IT], FP16, tag=f"avc{cc}", name=f"avc{it}_{cc}")
                   for cc in range(2)]
            for ib in range(4):
                rden = sm.tile([128, 1], FP32, tag="rden", name=f"rden{it}_{ib}")
                nc.vector.reciprocal(rden, av_t[ib][:, C:C + 1])
                avn = sm.tile([128, C], FP32, tag="avn", name=f"avn{it}_{ib}")
                nc.vector.tensor_scalar_mul(avn, in0=av_t[ib][:, 0:C], scalar1=rden)
                tp = avp.tile([128, C], FP32, tag="av", name=f"tp{it}_{ib}")
                nc.tensor.transpose(tp[:, 0:128], avn[:, 0:128], ident)
                nc.tensor.transpose(tp[:, 128:256], avn[:, 128:256], ident)
                for cc in range(2):
                    nc.vector.tensor_copy(avc[cc][:, 128 * ib:128 * (ib + 1)],
                                          tp[:, 128 * cc:128 * (cc + 1)])
            for ob in range(2):
                pj = avp.tile([128, IT], FP32, tag="av", name=f"pj{it}_{ob}")
                for cc in range(2):
                    nc.tensor.matmul(
                        pj, lhsT=wp_sb[cc][:, 128 * ob:128 * (ob + 1)],
                        rhs=avc[cc], start=(cc == 0), stop=(cc == 1))
                nc.vector.tensor_copy(proj_sb[ob][:, isl], pj)
                nc.vector.bn_stats(stats_sb[ob][:, it, :], proj_sb[ob][:, isl])

        # ---- main attention loop, software-pipelined across bursts -----
        # PE order per step: QK(next burst) then AV(current burst), so the
        # exp of the next burst (ACT) always overlaps AV matmuls (PE).
        pt_hold = emit_qk(0, 0)
        for it in range(N_ITILES):
            av_t = [avp.tile([128, C + 1], FP32, tag="av", name=f"av{it}_{ib}")
                    for ib in range(4)]
            for jb in range(N_JBURSTS):
                last = (it == N_ITILES - 1 and jb == N_JBURSTS - 1)
                if not last:
                    nit, njb = (it, jb + 1) if jb + 1 < N_JBURSTS else (it + 1, 0)
                    pt_next = emit_qk(nit, njb)
                else:
                    pt_next = None
                emit_av(av_t, jb, pt_hold)
                pt_hold = pt_next
            emit_epilogue(it, av_t)

        # ---- cross-core InstanceNorm stats -----------------------------
        ccin = persist.tile([128, 4], FP32, tag="ccin", name="ccin")
        for ob in range(2):
            mv = sm.tile([128, 2], FP32, tag="mv", name=f"mv{ob}")
            nc.vector.bn_aggr(out=mv, in_=stats_sb[ob])
            nc.vector.tensor_copy(ccin[:, 2 * ob:2 * (ob + 1)], mv)

        ccA = persist.tile([128, 4], FP32, tag="ccA", name="ccA")
        ccB = persist.tile([128, 4], FP32, tag="ccB", name="ccB")
        if use_collective:
            ccin_dr = dramp.tile([128, 4], FP32, tag="ccin_d", name="ccin_dr")
            ccout_dr = dramp.tile([2, 128, 4], FP32, tag="ccout_d", name="ccout_dr")
            nc.sync.dma_start(ccin_dr, ccin)
            nc.gpsimd.collective_compute(
                "AllGather", ALU.bypass, replica_groups=REPLICA_GROUPS,
                ins=[ccin_dr.opt()], outs=[ccout_dr.opt()])
            nc.sync.dma_start(ccA, ccout_dr[0])
            nc.sync.dma_start(ccB, ccout_dr[1])
        else:
            nc.vector.tensor_copy(ccA, ccin)
            nc.vector.tensor_copy(ccB, ccin)

        for ob in range(2):
            mA, vA = ccA[:, 2 * ob:2 * ob + 1], ccA[:, 2 * ob + 1:2 * ob + 2]
            mB, vB = ccB[:, 2 * ob:2 * ob + 1], ccB[:, 2 * ob + 1:2 * ob + 2]
            mean = sm.tile([128, 1], FP32, tag="mean", name=f"mean{ob}")
            nc.vector.tensor_add(mean, mA, mB)
            nc.vector.tensor_scalar_mul(mean, in0=mean, scalar1=0.5)
            d = sm.tile([128, 1], FP32, tag="d", name=f"d{ob}")
            nc.vector.tensor_sub(d, mA, mB)
            nc.vector.tensor_mul(d, d, d)
            var = sm.tile([128, 1], FP32, tag="var", name=f"var{ob}")
            nc.vector.tensor_add(var, vA, vB)
            nc.vector.tensor_scalar_mul(var, in0=var, scalar1=0.5)
            nc.vector.tensor_scalar_mul(d, in0=d, scalar1=0.25)
            nc.vector.tensor_add(var, var, d)
            # rstd = 1/sqrt(var + eps)
            rstd = sm.tile([128, 1], FP32, tag="rstd", name=f"rstd{ob}")
            nc.scalar.activation(out=rstd, in_=var, func=AF.Sqrt, bias=eps_sb,
                                 scale=1.0)
            nc.vector.reciprocal(rstd, rstd)
            # out = (proj - mean) * rstd + x1, chunked so the store DMAs
            # overlap the remaining vector work
            for ch in range(4):
                sl = slice(512 * ch, 512 * (ch + 1))
                nc.vector.tensor_scalar(
                    out=proj_sb[ob][:, sl], in0=proj_sb[ob][:, sl],
                    scalar1=mean, scalar2=rstd,
                    op0=ALU.subtract, op1=ALU.mult)
                nc.vector.tensor_add(proj_sb[ob][:, sl], proj_sb[ob][:, sl],
                                     x1_sb[ob][:, sl])
                nc.sync.dma_start(out_d[128 * ob:128 * (ob + 1), sl],
                                  proj_sb[ob][:, sl])


_nc_cache = None


def _get_nc():
    global _nc_cache
    if _nc_cache is None:
        _nc_cache = build_nc()
    return _nc_cache


def make_in_maps(x1, x2, wq, wk, wv, wp):
    x1f = np.ascontiguousarray(x1, dtype=np.float32).reshape(B, C, N)
    x2f = np.asarray(x2, np.float32).reshape(B, C, N).astype(np.float16)
    wqT = np.ascontiguousarray(((np.asarray(wq, np.float32) * SCALE).T).astype(np.float16))
    wkT = np.ascontiguousarray((np.asarray(wk, np.float32).T).astype(np.float16))
    wvT = np.ascontiguousarray((np.asarray(wv, np.float32).T).astype(np.float16))
    wpT = np.ascontiguousarray((np.asarray(wp, np.float32).T).astype(np.float16))
    in_maps = []
    for core in range(N_CORES):
        b, h = core // 2, core % 2
        in_maps.append({
            "x1h": np.ascontiguousarray(x1f[b, :, h * NH:(h + 1) * NH]),
            "x2b": np.ascontiguousarray(x2f[b]),
            "wqT": wqT, "wkT": wkT, "wvT": wvT, "wpT": wpT,
        })
    return in_maps


def assemble_out(results):
    out = np.empty((B, C, N), np.float32)
    for core in range(N_CORES):
        b, h = core // 2, core % 2
        out[b, :, h * NH:(h + 1) * NH] = results[core]["out"]
    return out.reshape(B, C, 16, 16, 16)


def kernel(**inputs):
    global LAST_RESULTS
    in_maps = make_in_maps(inputs["x1"], inputs["x2"], inputs["wq"],
                           inputs["wk"], inputs["wv"], inputs["wp"])
    res = run_bass_kernel_spmd(_get_nc(), in_maps, core_ids=list(range(N_CORES)))
    LAST_RESULTS = res
    return assemble_out(res.results)
